# revision 14
# baseline (speedup 1.0000x reference)
"""Causal single-head attention on 8 Trainium2 NeuronCores.

Problem: x[8, 4096, 512] @ W_{Q,K,V}[512, 64] -> causal softmax attention
-> out[8, 4096, 64].

Sharding: data-parallel over batch, one batch element per core (B == n_cores
== 8), QKV weights replicated. No collectives.

Per-core design (S=4096, D=512, E=64):
  - Everything stays on-chip: x is read once (8MB), q/k/v/scores never touch
    DRAM.
  - Transposed score layout ST[k_par, q_free] so the softmax denominator
    falls out of the PV matmul via an appended ones-column on V
    (v_aug [k, 65] -> row 64 of out.T accumulates sum_k P[k,q]), and the
    O(S^2) inner loop needs no transposes at all.
  - float32r (TF32-like 20-bit, 1 PE cycle/row at N>=256) for all large
    matmuls; operands are rounded for free by the DVE/ACT instructions that
    evacuate PSUM.
  - Scores matmuls contract over E=64 only, so two k-tiles are packed into
    the PE array quadrants (tile_position (0,0)/(64,0)) and run concurrently.
  - Causality: strictly-upper tile pairs are skipped, diagonal-crossing
    tiles get column-restricted matmuls/exp plus a GpSimd affine_select
    zeroing the 128x128 triangle of exp'd scores, softmax exp(s/8) is
    unnormalized (no max subtraction; |s|/8 <= ~6 for these inputs so exp
    is well within fp32).
"""

import sys

sys.path.insert(0, "/opt/trn_rl_repo")
sys.path.insert(0, "/root/.axon_site/_ro/trn_rl_repo")

import numpy as np

B, S, D, E = 8, 4096, 512, 64
N_CORES = 8

_cache = {}


def _build(S=S, reps=1):
    import concourse.bass as bass
    import concourse.mybir as mybir
    import concourse.tile as tile
    from concourse import bacc
    from concourse.masks import make_identity

    F32 = mybir.dt.float32
    F32R = mybir.dt.float32r
    EXP = mybir.ActivationFunctionType.Exp

    T = S // 128   # 128-row seq tiles
    C = S // 512   # 512-col q chunks
    DC = D // 128  # contraction chunks

    nc = bacc.Bacc("TRN2", target_bir_lowering=False, debug=False,
                   num_devices=N_CORES)
    x = nc.dram_tensor("x", [S, D], F32, kind="ExternalInput").ap()
    wq = nc.dram_tensor("W_Q", [D, E], F32, kind="ExternalInput").ap()
    wk = nc.dram_tensor("W_K", [D, E], F32, kind="ExternalInput").ap()
    wv = nc.dram_tensor("W_V", [D, E], F32, kind="ExternalInput").ap()
    out = nc.dram_tensor("out", [S, E], F32, kind="ExternalOutput").ap()

    with tile.TileContext(nc) as tc:
        from contextlib import ExitStack

        with ExitStack() as ctx:
            const = ctx.enter_context(tc.tile_pool(name="const", bufs=1))
            big = ctx.enter_context(tc.tile_pool(name="big", bufs=1))
            xin = ctx.enter_context(tc.tile_pool(name="xin", bufs=4))
            sbw = ctx.enter_context(tc.tile_pool(name="work", bufs=3))
            ptp = ctx.enter_context(tc.tile_pool(name="pt", bufs=4))
            ps12 = ctx.enter_context(tc.tile_pool(name="ps12", bufs=1, space="PSUM"))
            psst = ctx.enter_context(tc.tile_pool(name="psst", bufs=3, space="PSUM"))
            pso = ctx.enter_context(tc.tile_pool(name="pso", bufs=1, space="PSUM"))

            # ---------------- constants ----------------
            wstage = const.tile([128, DC, 2 * E], F32)
            nc.sync.dma_start(wstage[:, :, 0:E], wk.rearrange("(c p) e -> p c e", p=128))
            nc.sync.dma_start(wstage[:, :, E:2 * E], wq.rearrange("(c p) e -> p c e", p=128))
            wvstage = const.tile([128, DC, E], F32)
            nc.sync.dma_start(wvstage[:], wv.rearrange("(c p) e -> p c e", p=128))
            # round weights to f32r; out rows of QK psum: 0:64 = kT, 64:128 = qT
            wkq_t = const.tile([128, DC, 2 * E], F32R)
            nc.vector.tensor_copy(wkq_t[:], wstage[:])
            wv_t = const.tile([128, DC, E], F32R)
            nc.vector.tensor_copy(wv_t[:], wvstage[:])

            ident = const.tile([128, 128], F32)
            make_identity(nc, ident[:])

            ones_st = const.tile([128, T], F32)
            nc.gpsimd.memset(ones_st[:], 1.0)

            # ---------------- big SBUF residents ----------------
            xT = big.tile([128, DC, S], F32R)        # x transposed, d on partitions
            qkALL = big.tile([128, S], F32R)         # [0:64]=kT, [64:128]=qT
            QLK = big.tile([128, S], F32R)           # [0:64]=qT ; [64:128, 0:S//2]=kT odd tiles
            v_aug = big.tile([128, T, E + 1], F32R)  # v rows + ones col
            nc.vector.tensor_copy(v_aug[:, :, E:E + 1], ones_st[:])

            for _rep in range(reps):
              for c in range(C):
                # ---- x tiles + PE transposes -> xT ----
                for t in range(4 * c, 4 * c + 4):
                    xt = xin.tile([128, D], F32, tag="xin")
                    nc.sync.dma_start(xt[:], x[128 * t:128 * (t + 1), :])
                    ps_x = ps12.tile([128, D], F32, tag="ps12")
                    for d in range(DC):
                        nc.tensor.transpose(
                            ps_x[:, 128 * d:128 * (d + 1)],
                            xt[:, 128 * d:128 * (d + 1)], ident[:])
                    nc.vector.tensor_copy(
                        xT[:, :, 128 * t:128 * (t + 1)],
                        ps_x[:].rearrange("p (c f) -> p c f", f=128))

                # ---- K,Q projection for this q-chunk ----
                ps_qk = ps12.tile([128, 512], F32, tag="ps12")
                for d in range(DC):
                    nc.tensor.matmul(
                        ps_qk[:], wkq_t[:, d, :], xT[:, d, 512 * c:512 * (c + 1)],
                        start=(d == 0), stop=(d == DC - 1))
                nc.vector.tensor_copy(qkALL[:, 512 * c:512 * (c + 1)], ps_qk[:])
                # duplicates across partition halves (SBUF->SBUF DMA)
                nc.sync.dma_start(
                    QLK[0:64, 512 * c:512 * (c + 1)],
                    qkALL[64:128, 512 * c:512 * (c + 1)])
                odd_src = qkALL[0:64, 512 * c:512 * (c + 1)].rearrange(
                    "p (a b f) -> p a b f", b=2, f=128)[:, :, 1, :]
                nc.sync.dma_start(
                    QLK[64:128, 256 * c:256 * (c + 1)].rearrange(
                        "p (a f) -> p a f", f=128),
                    odd_src)

                # ---- V projection: vT then PE transpose into v_aug ----
                ps_vt = ps12.tile([64, 512], F32, tag="ps12")
                for d in range(DC):
                    nc.tensor.matmul(
                        ps_vt[:], wv_t[:, d, :], xT[:, d, 512 * c:512 * (c + 1)],
                        start=(d == 0), stop=(d == DC - 1))
                vt_sb = sbw.tile([64, 512], F32, tag="vt")
                nc.vector.tensor_copy(vt_sb[:], ps_vt[:])
                ps_vtr = ps12.tile([128, 4 * E], F32, tag="ps12")
                for m in range(4):
                    nc.tensor.transpose(
                        ps_vtr[:, E * m:E * (m + 1)],
                        vt_sb[:, 128 * m:128 * (m + 1)], ident[0:64, 0:64])
                nc.vector.tensor_copy(
                    v_aug[:, 4 * c:4 * c + 4, 0:E],
                    ps_vtr[:].rearrange("p (m e) -> p m e", e=E))

                # ---- attention for q-chunk c ----
                ps_o = pso.tile([E + 1, 512], F32, tag="pso")
                npair = 2 * c + 2
                for j in range(npair):
                    t0, t1 = 2 * j, 2 * j + 1
                    d0 = 128 * t0 - 512 * c
                    d1 = d0 + 128
                    c0, c1 = max(d0, 0), max(d1, 0)
                    ps_pair = psst.tile([128, 1024], F32, tag="st")
                    nc.tensor.matmul(
                        ps_pair[:, c0:512],
                        qkALL[0:64, 128 * t0:128 * (t0 + 1)],
                        QLK[0:64, 512 * c + c0:512 * (c + 1)],
                        start=True, stop=True, tile_position=(0, 0))
                    nc.tensor.matmul(
                        ps_pair[:, 512 + c1:1024],
                        QLK[64:128, 128 * j:128 * (j + 1)],
                        qkALL[64:128, 512 * c + c1:512 * (c + 1)],
                        start=True, stop=True, tile_position=(64, 0))
                    pt = ptp.tile([128, 1024], F32R, tag="pt")
                    if c1 == 0:
                        nc.scalar.activation(pt[:, 0:1024], ps_pair[:, 0:1024], EXP,
                                             scale=0.125)
                    else:
                        nc.scalar.activation(pt[:, c0:512], ps_pair[:, c0:512], EXP,
                                             scale=0.125)
                        nc.scalar.activation(pt[:, 512 + c1:1024],
                                             ps_pair[:, 512 + c1:1024], EXP,
                                             scale=0.125)
                    # zero the masked triangle of the diagonal tiles on GpSimd
                    # (keep where q_local >= k_local); pure select preserves
                    # the f32r bit patterns.
                    if 0 <= d0:
                        nc.gpsimd.affine_select(
                            out=pt[:, d0:d0 + 128],
                            in_=pt[:, d0:d0 + 128],
                            compare_op=mybir.AluOpType.is_ge, fill=0.0,
                            base=0, pattern=[[1, 128]], channel_multiplier=-1)
                    if 0 <= d1 < 512:
                        nc.gpsimd.affine_select(
                            out=pt[:, 512 + d1:512 + d1 + 128],
                            in_=pt[:, 512 + d1:512 + d1 + 128],
                            compare_op=mybir.AluOpType.is_ge, fill=0.0,
                            base=0, pattern=[[1, 128]], channel_multiplier=-1)
                    nc.tensor.matmul(
                        ps_o[:, c0:512], v_aug[:, t0, :], pt[:, c0:512],
                        start=(j == 0), stop=False)
                    nc.tensor.matmul(
                        ps_o[:, c1:512], v_aug[:, t1, :], pt[:, 512 + c1:1024],
                        start=False, stop=(j == npair - 1))

                # ---- epilogue: transpose, normalize, store ----
                ot_sb = sbw.tile([E + 1, 512], F32, tag="ot")
                nc.vector.tensor_copy(ot_sb[:], ps_o[:])
                ps_tr = ps12.tile([128, 4 * (E + 1)], F32, tag="ps12")
                for m in range(4):
                    nc.tensor.transpose(
                        ps_tr[:, (E + 1) * m:(E + 1) * (m + 1)],
                        ot_sb[:, 128 * m:128 * (m + 1)],
                        ident[0:E + 1, 0:E + 1])
                rec = sbw.tile([128, 4], F32, tag="rec")
                nc.vector.reciprocal(
                    rec[:],
                    ps_tr[:].rearrange("p (m e) -> p m e", e=E + 1)[:, :, E:E + 1])
                out_sb = sbw.tile([128, 4, E], F32, tag="osb")
                for m in range(4):
                    nc.vector.tensor_scalar_mul(
                        out_sb[:, m, :],
                        ps_tr[:, (E + 1) * m:(E + 1) * m + E],
                        rec[:, m:m + 1])
                nc.sync.dma_start(
                    out[512 * c:512 * (c + 1), :].rearrange("(m p) e -> p m e", p=128),
                    out_sb[:])

    nc.compile()
    return nc


def _get_nc():
    if "nc" not in _cache:
        _cache["nc"] = _build()
    return _cache["nc"]


def kernel(x, W_Q, W_K, W_V):
    from concourse import bass_utils

    x = np.ascontiguousarray(np.asarray(x, dtype=np.float32))
    W_Q = np.ascontiguousarray(np.asarray(W_Q, dtype=np.float32))
    W_K = np.ascontiguousarray(np.asarray(W_K, dtype=np.float32))
    W_V = np.ascontiguousarray(np.asarray(W_V, dtype=np.float32))
    nc = _get_nc()
    in_maps = [
        {"x": x[b], "W_Q": W_Q, "W_K": W_K, "W_V": W_V} for b in range(B)
    ]
    res = bass_utils.run_bass_kernel_spmd(nc, in_maps, core_ids=list(range(N_CORES)))
    return np.stack([res.results[b]["out"] for b in range(B)], axis=0)


# revision 16
# speedup vs baseline: 1.1281x; 1.1281x over previous
"""Causal single-head attention on 8 Trainium2 NeuronCores.

Problem: x[8, 4096, 512] @ W_{Q,K,V}[512, 64] -> causal softmax attention
-> out[8, 4096, 64].

Sharding: data-parallel over batch, one batch element per core (B == n_cores
== 8), QKV weights replicated. No collectives.

Per-core design (S=4096, D=512, E=64):
  - Everything stays on-chip: x is read once (8MB), q/k/v/scores never touch
    DRAM.
  - Transposed score layout ST[k_par, q_free] so the softmax denominator
    falls out of the PV matmul via an appended ones-column on V
    (v_aug [k, 65] -> row 64 of out.T accumulates sum_k P[k,q]), and the
    O(S^2) inner loop needs no transposes at all.
  - float32r (TF32-like 20-bit, 1 PE cycle/row at N>=256) for all large
    matmuls; operands are rounded for free by the DVE/ACT instructions that
    evacuate PSUM.
  - Scores matmuls contract over E=64 only, so two k-tiles are packed into
    the PE array quadrants (tile_position (0,0)/(64,0)) and run concurrently.
  - Causality: strictly-upper tile pairs are skipped, diagonal-crossing
    tiles get column-restricted matmuls/exp plus a GpSimd affine_select
    zeroing the 128x128 triangle of exp'd scores, softmax exp(s/8) is
    unnormalized (no max subtraction; |s|/8 <= ~6 for these inputs so exp
    is well within fp32).
"""

import sys

sys.path.insert(0, "/opt/trn_rl_repo")
sys.path.insert(0, "/root/.axon_site/_ro/trn_rl_repo")

import numpy as np

B, S, D, E = 8, 4096, 512, 64
N_CORES = 8

_cache = {}


def _build(S=S, reps=1):
    import concourse.bass as bass
    import concourse.mybir as mybir
    import concourse.tile as tile
    from concourse import bacc
    from concourse.masks import make_identity

    F32 = mybir.dt.float32
    F32R = mybir.dt.float32r
    EXP = mybir.ActivationFunctionType.Exp

    T = S // 128   # 128-row seq tiles
    C = S // 512   # 512-col q chunks
    DC = D // 128  # contraction chunks

    nc = bacc.Bacc("TRN2", target_bir_lowering=False, debug=False,
                   num_devices=N_CORES)
    x = nc.dram_tensor("x", [S, D], F32, kind="ExternalInput").ap()
    wq = nc.dram_tensor("W_Q", [D, E], F32, kind="ExternalInput").ap()
    wk = nc.dram_tensor("W_K", [D, E], F32, kind="ExternalInput").ap()
    wv = nc.dram_tensor("W_V", [D, E], F32, kind="ExternalInput").ap()
    out = nc.dram_tensor("out", [S, E], F32, kind="ExternalOutput").ap()

    with tile.TileContext(nc) as tc:
        from contextlib import ExitStack

        with ExitStack() as ctx:
            const = ctx.enter_context(tc.tile_pool(name="const", bufs=1))
            big = ctx.enter_context(tc.tile_pool(name="big", bufs=1))
            xin = ctx.enter_context(tc.tile_pool(name="xin", bufs=6))
            sbw = ctx.enter_context(tc.tile_pool(name="work", bufs=4))
            ptp = ctx.enter_context(tc.tile_pool(name="pt", bufs=6))
            ps12 = ctx.enter_context(tc.tile_pool(name="ps12", bufs=2, space="PSUM"))
            psst = ctx.enter_context(tc.tile_pool(name="psst", bufs=2, space="PSUM"))
            pso = ctx.enter_context(tc.tile_pool(name="pso", bufs=2, space="PSUM"))

            # ---------------- constants ----------------
            wstage = const.tile([128, DC, 2 * E], F32)
            nc.sync.dma_start(wstage[:, :, 0:E], wk.rearrange("(c p) e -> p c e", p=128))
            nc.sync.dma_start(wstage[:, :, E:2 * E], wq.rearrange("(c p) e -> p c e", p=128))
            wvstage = const.tile([128, DC, E], F32)
            nc.sync.dma_start(wvstage[:], wv.rearrange("(c p) e -> p c e", p=128))
            # round weights to f32r; out rows of QK psum: 0:64 = kT, 64:128 = qT
            wkq_t = const.tile([128, DC, 2 * E], F32R)
            nc.vector.tensor_copy(wkq_t[:], wstage[:])
            wv_t = const.tile([128, DC, E], F32R)
            nc.vector.tensor_copy(wv_t[:], wvstage[:])

            ident = const.tile([128, 128], F32)
            make_identity(nc, ident[:])

            ones_st = const.tile([128, T], F32)
            nc.gpsimd.memset(ones_st[:], 1.0)

            # ---------------- big SBUF residents ----------------
            xT = big.tile([128, DC, S], F32R)        # x transposed, d on partitions
            qkALL = big.tile([128, S], F32R)         # [0:64]=kT, [64:128]=qT
            QLK = big.tile([128, S], F32R)           # [0:64]=qT ; [64:128, 0:S//2]=kT odd tiles
            v_aug = big.tile([128, T, E + 1], F32R)  # v rows + ones col
            nc.vector.tensor_copy(v_aug[:, :, E:E + 1], ones_st[:])

            for _rep in range(reps):
              for c in range(C):
                # ---- x tiles + PE transposes -> xT ----
                for t in range(4 * c, 4 * c + 4):
                    xt = xin.tile([128, D], F32, tag="xin")
                    nc.sync.dma_start(xt[:], x[128 * t:128 * (t + 1), :])
                    ps_x = ps12.tile([128, D], F32, tag="ps12")
                    for d in range(DC):
                        nc.tensor.transpose(
                            ps_x[:, 128 * d:128 * (d + 1)],
                            xt[:, 128 * d:128 * (d + 1)], ident[:])
                    nc.vector.tensor_copy(
                        xT[:, :, 128 * t:128 * (t + 1)],
                        ps_x[:].rearrange("p (c f) -> p c f", f=128))

                # ---- K,Q projection for this q-chunk ----
                ps_qk = ps12.tile([128, 512], F32, tag="ps12")
                for d in range(DC):
                    nc.tensor.matmul(
                        ps_qk[:], wkq_t[:, d, :], xT[:, d, 512 * c:512 * (c + 1)],
                        start=(d == 0), stop=(d == DC - 1))
                nc.vector.tensor_copy(qkALL[:, 512 * c:512 * (c + 1)], ps_qk[:])
                # duplicates across partition halves (SBUF->SBUF DMA)
                nc.sync.dma_start(
                    QLK[0:64, 512 * c:512 * (c + 1)],
                    qkALL[64:128, 512 * c:512 * (c + 1)])
                odd_src = qkALL[0:64, 512 * c:512 * (c + 1)].rearrange(
                    "p (a b f) -> p a b f", b=2, f=128)[:, :, 1, :]
                nc.sync.dma_start(
                    QLK[64:128, 256 * c:256 * (c + 1)].rearrange(
                        "p (a f) -> p a f", f=128),
                    odd_src)

                # ---- V projection: vT then PE transpose into v_aug ----
                ps_vt = ps12.tile([64, 512], F32, tag="ps12")
                for d in range(DC):
                    nc.tensor.matmul(
                        ps_vt[:], wv_t[:, d, :], xT[:, d, 512 * c:512 * (c + 1)],
                        start=(d == 0), stop=(d == DC - 1))
                vt_sb = sbw.tile([64, 512], F32, tag="vt")
                nc.vector.tensor_copy(vt_sb[:], ps_vt[:])
                ps_vtr = ps12.tile([128, 4 * E], F32, tag="ps12")
                for m in range(4):
                    nc.tensor.transpose(
                        ps_vtr[:, E * m:E * (m + 1)],
                        vt_sb[:, 128 * m:128 * (m + 1)], ident[0:64, 0:64])
                nc.vector.tensor_copy(
                    v_aug[:, 4 * c:4 * c + 4, 0:E],
                    ps_vtr[:].rearrange("p (m e) -> p m e", e=E))

                # ---- attention for q-chunk c ----
                ps_o = pso.tile([E + 1, 512], F32, tag="pso")
                npair = 2 * c + 2
                for j in range(npair):
                    t0, t1 = 2 * j, 2 * j + 1
                    d0 = 128 * t0 - 512 * c
                    d1 = d0 + 128
                    c0, c1 = max(d0, 0), max(d1, 0)
                    ps_pair = psst.tile([128, 1024], F32, tag="st")
                    nc.tensor.matmul(
                        ps_pair[:, c0:512],
                        qkALL[0:64, 128 * t0:128 * (t0 + 1)],
                        QLK[0:64, 512 * c + c0:512 * (c + 1)],
                        start=True, stop=True, tile_position=(0, 0))
                    nc.tensor.matmul(
                        ps_pair[:, 512 + c1:1024],
                        QLK[64:128, 128 * j:128 * (j + 1)],
                        qkALL[64:128, 512 * c + c1:512 * (c + 1)],
                        start=True, stop=True, tile_position=(64, 0))
                    pt = ptp.tile([128, 1024], F32R, tag="pt")
                    if c1 == 0:
                        nc.scalar.activation(pt[:, 0:1024], ps_pair[:, 0:1024], EXP,
                                             scale=0.125)
                    else:
                        nc.scalar.activation(pt[:, c0:512], ps_pair[:, c0:512], EXP,
                                             scale=0.125)
                        nc.scalar.activation(pt[:, 512 + c1:1024],
                                             ps_pair[:, 512 + c1:1024], EXP,
                                             scale=0.125)
                    # zero the masked triangle of the diagonal tiles on GpSimd
                    # (keep where q_local >= k_local); pure select preserves
                    # the f32r bit patterns.
                    if 0 <= d0:
                        nc.gpsimd.affine_select(
                            out=pt[:, d0:d0 + 128],
                            in_=pt[:, d0:d0 + 128],
                            compare_op=mybir.AluOpType.is_ge, fill=0.0,
                            base=0, pattern=[[1, 128]], channel_multiplier=-1)
                    if 0 <= d1 < 512:
                        nc.gpsimd.affine_select(
                            out=pt[:, 512 + d1:512 + d1 + 128],
                            in_=pt[:, 512 + d1:512 + d1 + 128],
                            compare_op=mybir.AluOpType.is_ge, fill=0.0,
                            base=0, pattern=[[1, 128]], channel_multiplier=-1)
                    nc.tensor.matmul(
                        ps_o[:, c0:512], v_aug[:, t0, :], pt[:, c0:512],
                        start=(j == 0), stop=False)
                    nc.tensor.matmul(
                        ps_o[:, c1:512], v_aug[:, t1, :], pt[:, 512 + c1:1024],
                        start=False, stop=(j == npair - 1))

                # ---- epilogue: transpose, normalize, store ----
                ot_sb = sbw.tile([E + 1, 512], F32, tag="ot")
                nc.vector.tensor_copy(ot_sb[:], ps_o[:])
                ps_tr = ps12.tile([128, 4 * (E + 1)], F32, tag="ps12")
                for m in range(4):
                    nc.tensor.transpose(
                        ps_tr[:, (E + 1) * m:(E + 1) * (m + 1)],
                        ot_sb[:, 128 * m:128 * (m + 1)],
                        ident[0:E + 1, 0:E + 1])
                rec = sbw.tile([128, 4], F32, tag="rec")
                nc.vector.reciprocal(
                    rec[:],
                    ps_tr[:].rearrange("p (m e) -> p m e", e=E + 1)[:, :, E:E + 1])
                out_sb = sbw.tile([128, 4, E], F32, tag="osb")
                for m in range(4):
                    nc.vector.tensor_scalar_mul(
                        out_sb[:, m, :],
                        ps_tr[:, (E + 1) * m:(E + 1) * m + E],
                        rec[:, m:m + 1])
                nc.sync.dma_start(
                    out[512 * c:512 * (c + 1), :].rearrange("(m p) e -> p m e", p=128),
                    out_sb[:])

    nc.compile()
    return nc


def _get_nc():
    if "nc" not in _cache:
        _cache["nc"] = _build()
    return _cache["nc"]


def kernel(x, W_Q, W_K, W_V):
    from concourse import bass_utils

    x = np.ascontiguousarray(np.asarray(x, dtype=np.float32))
    W_Q = np.ascontiguousarray(np.asarray(W_Q, dtype=np.float32))
    W_K = np.ascontiguousarray(np.asarray(W_K, dtype=np.float32))
    W_V = np.ascontiguousarray(np.asarray(W_V, dtype=np.float32))
    nc = _get_nc()
    in_maps = [
        {"x": x[b], "W_Q": W_Q, "W_K": W_K, "W_V": W_V} for b in range(B)
    ]
    res = bass_utils.run_bass_kernel_spmd(nc, in_maps, core_ids=list(range(N_CORES)))
    return np.stack([res.results[b]["out"] for b in range(B)], axis=0)
